# revision 9
# baseline (speedup 1.0000x reference)
"""SparseNodeLinear on 8 Trainium2 NeuronCores (Bass/Tile).

Reference math:
    xt  = transpose(x, (0, 2, 1))                  # [B, N, K]
    xm  = where(edge_mask[None], 0, xt)
    ans = einsum('bnk,nkf->bnf', xm, weight)       # per-node matmul
    out = transpose(ans, (0, 2, 1)) + bias[None]   # [B, F, N]

Sharding: the node dim N=2048 is split across 8 cores (256 nodes each);
x is sliced along its last (node) axis so every input byte is read once.

Per-core compute: nodes are processed in 16 groups of 16. For one group,
a block matmul packs (node i, batch b) into the stationary M dim and
(node i', feature f) into the moving free dim:
    lhsT [K=128, M=128]: column m = i*8+b  holds masked x[b, kchunk, node i]
    rhs  [K=128, 512]:   column j = i'*32+f holds weight[node i', kchunk, f]
    psum[m, j] += lhsT.T @ rhs   accumulated over 16 k-chunks of 128
plus one final K=2 matmul appending the k=2048 row and an all-ones row
whose rhs carries the bias (so bias lands in PSUM for free). The useful
result is the block diagonal psum[8i:8i+8, 32i:32i+32]. Engine access
patterns must start at partition 0/32/64/96, so the diagonal cannot be
copied out directly; instead the PSUM block is copied to SBUF and 16
tiny matmuls against identity columns (lhsT = I[:, 8i:8i+8]) gather all
diagonal blocks into one partition-aligned [8, 512] PSUM tile (exact:
selection by 1.0/0.0). Off-diagonal work is wasted, but the kernel is
HBM-bound (reading weight dominates), so PE utilization is not the
limiter.

Host-side work is layout only: slicing/transposing inputs into
contiguous per-core DMA-friendly arrays and unshuffling the output.
"""

import numpy as np

B = 8          # batch
N = 2048       # nodes
K = 2049       # contraction length (N+1)
F = 32         # out features
NCORES = 8
NLOC = N // NCORES   # 256 nodes per core
NG = 16              # nodes per group
G = NLOC // NG       # 16 groups per core
KC = 16              # k-chunks of 128 covering k < 2048
P = 128

_NC_CACHE = {}


def _build_bass():
    import concourse.mybir as mybir
    from concourse import bacc
    from concourse.tile import TileContext

    f32 = mybir.dt.float32
    nc = bacc.Bacc("TRN2", target_bir_lowering=False, debug=False,
                   num_devices=NCORES)

    wm_d = nc.dram_tensor("wm", [G, P, KC, NG, F], f32, kind="ExternalInput")
    wl_d = nc.dram_tensor("wl", [G, 2, NG, F], f32, kind="ExternalInput")
    xm_d = nc.dram_tensor("xm", [G, P, KC, NG, B], f32, kind="ExternalInput")
    xl_d = nc.dram_tensor("xl", [G, 2, NG, B], f32, kind="ExternalInput")
    mf_d = nc.dram_tensor("mf", [P, KC, NLOC], f32, kind="ExternalInput")
    id_d = nc.dram_tensor("ident", [P, P], f32, kind="ExternalInput")
    out_d = nc.dram_tensor("out", [G, B, NG * F], f32, kind="ExternalOutput")

    wm, wl, xm, xl, mf, ident, out = (t.ap() for t in
                                      (wm_d, wl_d, xm_d, xl_d, mf_d,
                                       id_d, out_d))

    with TileContext(nc) as tc:
        with (
            tc.tile_pool(name="wpool", bufs=3) as wpool,
            tc.tile_pool(name="xpool", bufs=3) as xpool,
            tc.tile_pool(name="mpool", bufs=3) as mpool,
            tc.tile_pool(name="spool", bufs=3) as spool,
            tc.tile_pool(name="cpool", bufs=1) as cpool,
            tc.tile_pool(name="dpool", bufs=3) as dpool,
            tc.tile_pool(name="ppool", bufs=4, space="PSUM") as ppool,
            tc.tile_pool(name="p2pool", bufs=2, space="PSUM") as p2pool,
        ):
            mf_t = cpool.tile([P, KC, NLOC], f32, tag="mf")
            nc.sync.dma_start(mf_t[:], mf[:])
            id_t = cpool.tile([P, P], f32, tag="ident")
            nc.sync.dma_start(id_t[:], ident[:])

            for g in range(G):
                w_t = wpool.tile([P, KC, NG, F], f32, tag="w")
                nc.sync.dma_start(w_t[:], wm[g])
                wl_t = spool.tile([2, NG, F], f32, tag="wl")
                nc.sync.dma_start(wl_t[:], wl[g])
                x_t = xpool.tile([P, KC, NG, B], f32, tag="x")
                nc.sync.dma_start(x_t[:], xm[g])
                xl_t = spool.tile([2, NG, B], f32, tag="xl")
                nc.sync.dma_start(xl_t[:], xl[g])

                q_t = mpool.tile([P, KC, NG, B], f32, tag="q")
                m_sl = (mf_t[:, :, NG * g: NG * (g + 1)]
                        .unsqueeze(3).broadcast_to((P, KC, NG, B)))
                nc.vector.tensor_mul(q_t[:], x_t[:], m_sl)

                ps = ppool.tile([P, F * NG], f32, tag="ps")
                for kc in range(KC):
                    nc.tensor.matmul(ps[:], q_t[:, kc], w_t[:, kc],
                                     start=(kc == 0), stop=False)
                nc.tensor.matmul(ps[:], xl_t[:], wl_t[:],
                                 start=False, stop=True)

                sb_t = dpool.tile([P, NG * F], f32, tag="sb")
                nc.scalar.copy(sb_t[:], ps[:])

                ps2 = p2pool.tile([B, NG * F], f32, tag="ps2")
                for i in range(NG):
                    nc.tensor.matmul(ps2[:, F * i: F * (i + 1)],
                                     id_t[:, B * i: B * (i + 1)],
                                     sb_t[:, F * i: F * (i + 1)],
                                     start=(i == 0), stop=(i == NG - 1))

                og_t = spool.tile([B, NG * F], f32, tag="og")
                nc.vector.tensor_copy(og_t[:], ps2[:])
                nc.sync.dma_start(out[g], og_t[:])

    nc.compile()
    return nc


def get_nc():
    if "nc" not in _NC_CACHE:
        _NC_CACHE["nc"] = _build_bass()
    return _NC_CACHE["nc"]


def prep_core(x, weight, bias, notm, c):
    """Slice + relayout the full inputs into core c's contiguous arrays."""
    ns, ne = c * NLOC, (c + 1) * NLOC
    w = weight[ns:ne]                                   # [256, 2049, 32]
    wm = np.ascontiguousarray(
        w[:, : KC * P, :].reshape(G, NG, KC, P, F).transpose(0, 3, 2, 1, 4))
    wlast = w[:, K - 1, :].reshape(G, NG, F)
    bl = bias[:, ns:ne].T.reshape(G, NG, F)
    wl = np.ascontiguousarray(np.stack([wlast, bl], axis=1))

    xs = x[:, : KC * P, ns:ne]                          # [8, 2048, 256]
    xm = np.ascontiguousarray(
        xs.reshape(B, KC, P, G, NG).transpose(3, 2, 1, 4, 0))
    xlast = (x[:, K - 1, ns:ne] * notm[ns:ne, K - 1][None, :]).T
    xl = np.ascontiguousarray(
        np.stack([xlast.reshape(G, NG, B),
                  np.ones((G, NG, B), np.float32)], axis=1))

    mf = np.ascontiguousarray(
        notm[ns:ne, : KC * P].T.reshape(KC, P, NLOC).transpose(1, 0, 2))

    return {"wm": wm, "wl": wl, "xm": xm, "xl": xl, "mf": mf,
            "ident": np.eye(P, dtype=np.float32)}


def unshuffle_out(raw):
    """Per-core device output [G, B, NG*F] -> [B, F, NLOC]."""
    return (raw.reshape(G, B, NG, F).transpose(1, 3, 0, 2)
            .reshape(B, F, NLOC))


def kernel(x, weight, bias, edge_mask):
    from concourse.bass_utils import run_bass_kernel_spmd

    x = np.ascontiguousarray(x, dtype=np.float32)
    weight = np.ascontiguousarray(weight, dtype=np.float32)
    bias = np.ascontiguousarray(bias, dtype=np.float32)
    notm = (~np.asarray(edge_mask)).astype(np.float32)

    nc = get_nc()
    in_maps = [prep_core(x, weight, bias, notm, c) for c in range(NCORES)]
    res = run_bass_kernel_spmd(nc, in_maps, core_ids=list(range(NCORES)))

    ans = np.empty((B, F, N), np.float32)
    for c in range(NCORES):
        ans[:, :, c * NLOC:(c + 1) * NLOC] = unshuffle_out(res.results[c]["out"])
    return ans


# revision 19
# speedup vs baseline: 262.2653x; 262.2653x over previous
"""SparseNodeLinear on 8 Trainium2 NeuronCores (Bass/Tile).

Reference math:
    xt  = transpose(x, (0, 2, 1))                  # [B, N, K]
    xm  = where(edge_mask[None], 0, xt)
    ans = einsum('bnk,nkf->bnf', xm, weight)       # per-node matmul
    out = transpose(ans, (0, 2, 1)) + bias[None]   # [B, F, N]

Sharding: the node dim N=2048 is split across 8 cores (256 nodes each);
x is sliced along its last (node) axis so every input byte is read once.

Per-core compute: nodes are processed in 16 groups of 16. For one group,
a block matmul packs (node i, batch b) into the stationary M dim and
(node i', feature f) into the moving free dim:
    lhsT [K=128, M=128]: column m = i*8+b  holds masked x[b, kchunk, node i]
    rhs  [K=128, 512]:   column j = i'*32+f holds weight[node i', kchunk, f]
    psum[m, j] += lhsT.T @ rhs   accumulated over 16 k-chunks of 128
plus one final K=2 matmul appending the k=2048 row and an all-ones row
whose rhs carries the bias (so bias lands in PSUM for free). Matmuls run
in plain float32 (exact; measured as fast as float32r on hardware here
since the kernel is bound by reading `weight` once per core).

The useful result is the block diagonal psum[8i:8i+8, 32i:32i+32].
Engine access patterns must start at partition 0/32/64/96, so the
diagonal cannot be copied out directly; instead the PSUM block is copied
to SBUF and 16 tiny matmuls against identity columns (lhsT =
I[:, 8i:8i+8]) gather all diagonal blocks into one partition-aligned
[8, 512] PSUM tile (exact: selection by 1.0/0.0 in plain fp32).

The edge mask rides along as a 9th "batch" column of the x tensor
(host packs x and the 0/1 mask into one [.., NG, 9] array), so masking
is a single broadcast multiply per group and needs no extra DMA.

Host-side work is layout only: slicing/transposing inputs into
contiguous per-core DMA-friendly arrays and unshuffling the output.
"""

import numpy as np

B = 8          # batch
N = 2048       # nodes
K = 2049       # contraction length (N+1)
F = 32         # out features
NCORES = 8
NLOC = N // NCORES   # 256 nodes per core
NG = 16              # nodes per group
G = NLOC // NG       # 16 groups per core
KC = 16              # k-chunks of 128 covering k < 2048
P = 128

_NC_CACHE = {}


def _build_bass(repeat=1, mmdt="float32"):
    # repeat>1 re-runs the whole body on-device; used only for timing
    # (wall-clock slope vs repeat amortizes the fixed dispatch overhead).
    # mmdt: dtype for the main matmul operands ("float32" is exact;
    # "float32r" streams 4x faster but rounds to ~tf32 precision).
    import concourse.mybir as mybir
    from concourse import bacc
    from concourse.tile import TileContext

    f32 = mybir.dt.float32
    mdt = getattr(mybir.dt, mmdt)
    nc = bacc.Bacc("TRN2", target_bir_lowering=False, debug=False,
                   num_devices=NCORES)

    wm_d = nc.dram_tensor("wm", [G, P, KC, NG, F], mdt, kind="ExternalInput")
    wl_d = nc.dram_tensor("wl", [G, 2, NG, F], mdt, kind="ExternalInput")
    xm_d = nc.dram_tensor("xm", [G, P, KC, NG, B + 1], f32,
                          kind="ExternalInput")
    xl_d = nc.dram_tensor("xl", [G, 2, NG, B], mdt, kind="ExternalInput")
    id_d = nc.dram_tensor("ident", [P, P], f32, kind="ExternalInput")
    out_d = nc.dram_tensor("out", [G, B, NG * F], f32, kind="ExternalOutput")

    wm, wl, xm, xl, ident, out = (t.ap() for t in
                                  (wm_d, wl_d, xm_d, xl_d, id_d, out_d))

    with TileContext(nc) as tc:
        with (
            tc.tile_pool(name="wpool", bufs=3) as wpool,
            tc.tile_pool(name="xpool", bufs=4) as xpool,
            tc.tile_pool(name="mpool", bufs=4) as mpool,
            tc.tile_pool(name="spool", bufs=3) as spool,
            tc.tile_pool(name="cpool", bufs=1) as cpool,
            tc.tile_pool(name="dpool", bufs=3) as dpool,
            tc.tile_pool(name="ppool", bufs=4, space="PSUM") as ppool,
            tc.tile_pool(name="p2pool", bufs=2, space="PSUM") as p2pool,
        ):
            id_t = cpool.tile([P, P], f32, tag="ident")
            nc.sync.dma_start(id_t[:], ident[:])

            def extract(g, ps):
                # gather the block diagonal of ps into an aligned [B, 512]
                # psum tile via identity-column matmuls, then store.
                sb_t = dpool.tile([P, NG * F], f32, tag="sb", name="sb_t")
                nc.scalar.copy(sb_t[:], ps[:])
                ps2 = p2pool.tile([B, NG * F], f32, tag="ps2", name="ps2")
                for i in range(NG):
                    nc.tensor.matmul(ps2[:, F * i: F * (i + 1)],
                                     id_t[:, B * i: B * (i + 1)],
                                     sb_t[:, F * i: F * (i + 1)],
                                     start=(i == 0), stop=(i == NG - 1))
                og_t = spool.tile([B, NG * F], f32, tag="og", name="og_t")
                nc.vector.tensor_copy(og_t[:], ps2[:])
                nc.sync.dma_start(out[g], og_t[:])

            pending = None
            for g in [g for _ in range(repeat) for g in range(G)]:
                x_t = xpool.tile([P, KC, NG, B + 1], f32, tag="x")
                nc.sync.dma_start(x_t[:], xm[g])
                w_t = wpool.tile([P, KC, NG, F], mdt, tag="w")
                nc.sync.dma_start(w_t[:], wm[g])
                wl_t = spool.tile([2, NG, F], mdt, tag="wl")
                nc.sync.dma_start(wl_t[:], wl[g])
                xl_t = spool.tile([2, NG, B], mdt, tag="xl")
                nc.sync.dma_start(xl_t[:], xl[g])

                q_t = mpool.tile([P, KC, NG, B], mdt, tag="q")
                m_sl = (x_t[:, :, :, B: B + 1]
                        .broadcast_to((P, KC, NG, B)))
                nc.vector.tensor_mul(q_t[:], x_t[:, :, :, 0: B], m_sl)

                ps = ppool.tile([P, NG * F], f32, tag="ps")
                for kc in range(KC):
                    nc.tensor.matmul(ps[:], q_t[:, kc], w_t[:, kc],
                                     start=(kc == 0), stop=False)
                nc.tensor.matmul(ps[:], xl_t[:], wl_t[:],
                                 start=False, stop=True)

                # extraction for the previous group lands here, between the
                # main matmul bursts, so its latency is off the critical path
                if pending is not None:
                    extract(*pending)
                pending = (g, ps)
            extract(*pending)

    nc.compile()
    return nc


def get_nc(repeat=1, mmdt="float32"):
    key = (repeat, mmdt)
    if key not in _NC_CACHE:
        _NC_CACHE[key] = _build_bass(repeat, mmdt)
    return _NC_CACHE[key]


def prep_core(x, weight, bias, notm, c):
    """Slice + relayout the full inputs into core c's contiguous arrays."""
    ns, ne = c * NLOC, (c + 1) * NLOC
    w = weight[ns:ne]                                   # [256, 2049, 32]
    wm = np.ascontiguousarray(
        w[:, : KC * P, :].reshape(G, NG, KC, P, F).transpose(0, 3, 2, 1, 4))
    wlast = w[:, K - 1, :].reshape(G, NG, F)
    bl = bias[:, ns:ne].T.reshape(G, NG, F)
    wl = np.ascontiguousarray(np.stack([wlast, bl], axis=1))  # [G,2,NG,F]

    xs = x[:, : KC * P, ns:ne]                          # [8, 2048, 256]
    xg = xs.reshape(B, KC, P, G, NG).transpose(3, 2, 1, 4, 0)
    mg = (notm[ns:ne, : KC * P].reshape(G, NG, KC, P)
          .transpose(0, 3, 2, 1))                       # [G, P, KC, NG]
    xm = np.ascontiguousarray(
        np.concatenate([xg, mg[..., None]], axis=-1))   # [G,P,KC,NG,9]

    xlast = (x[:, K - 1, ns:ne] * notm[ns:ne, K - 1][None, :]).T
    xl = np.ascontiguousarray(
        np.stack([xlast.reshape(G, NG, B),
                  np.ones((G, NG, B), np.float32)], axis=1))  # [G,2,NG,B]

    return {"wm": wm, "wl": wl, "xm": xm, "xl": xl,
            "ident": np.eye(P, dtype=np.float32)}


def unshuffle_out(raw):
    """Per-core device output [G, B, NG*F] -> [B, F, NLOC]."""
    return (raw.reshape(G, B, NG, F).transpose(1, 3, 0, 2)
            .reshape(B, F, NLOC))


def kernel(x, weight, bias, edge_mask):
    from concourse.bass_utils import run_bass_kernel_spmd

    x = np.ascontiguousarray(x, dtype=np.float32)
    weight = np.ascontiguousarray(weight, dtype=np.float32)
    bias = np.ascontiguousarray(bias, dtype=np.float32)
    notm = (~np.asarray(edge_mask)).astype(np.float32)

    nc = get_nc()
    in_maps = [prep_core(x, weight, bias, notm, c) for c in range(NCORES)]
    res = run_bass_kernel_spmd(nc, in_maps, core_ids=list(range(NCORES)))

    ans = np.empty((B, F, N), np.float32)
    for c in range(NCORES):
        ans[:, :, c * NLOC:(c + 1) * NLOC] = unshuffle_out(res.results[c]["out"])
    return ans
